# revision 5
# baseline (speedup 1.0000x reference)
"""Trainium2 Bass kernel for the Elman RNN problem (T=512, B=64, I=512, L=1024).

Strategy: data-parallel over batch across 8 NeuronCores (B=64 -> 8 per core).
Per core:
  phase 1: xi = x @ Wi^T + bi as one GEMM (bf16 operands, fp32 PSUM accumulate),
           bias folded in as a K=1 matmul row; result stored bf16 in DRAM.
  phase 2: 512 sequential steps of h = tanh(xi_t + h @ Wh^T).
           The step matmul streams Wh^T (moving operand, N=512 slices) against
           tiny stationary h^T chunks [128, 8], so the weight-load port is
           never the bottleneck. PSUM is initialized with xi_t via an identity
           matmul. tanh on ScalarE produces the fp32 output row directly; the
           bf16 h is re-transposed to [L, B]-major via 8 PE transposes for the
           next step's stationary operand.
"""
import os
import sys
if "/opt/trn_rl_repo" not in sys.path:
    sys.path.insert(0, "/opt/trn_rl_repo")

import numpy as np
import ml_dtypes

import concourse.bass as bass
import concourse.mybir as mybir
from concourse.tile import TileContext

F32 = mybir.dt.float32
BF16 = mybir.dt.bfloat16
BF16_NP = ml_dtypes.bfloat16

T, B, I, L = 512, 64, 512, 1024
T = int(os.environ.get("BASS_RNN_T", T))  # dev-only override; grading uses 512
NCORES = 8
BL = B // NCORES          # 8 batch rows per core
TB = T * BL               # 4096 rows of the phase-1 GEMM
NL = L // 128             # 8 chunks of the hidden dim
NI = I // 128             # 4 chunks of the input dim
WIN = 16                  # xi prefetch window (steps)


def _legalize_sync_waits(nc, max_waits=1):
    # Walrus in this toolchain encodes at most one sem-wait per instruction;
    # spill extra waits onto preceding same-engine NOPs.
    n_split = 0
    for fn in nc.m.functions:
        for bb in fn.blocks:
            out = []
            for inst in bb.instructions:
                si = inst.sync_info
                if si is not None and len(si.on_wait) > max_waits:
                    waits = list(si.on_wait)
                    for w in waits[:-max_waits]:
                        out.append(mybir.InstNoOp(
                            name=f"{inst.name}.w{n_split}",
                            engine=inst.engine,
                            ins=[], outs=[],
                            sync_info=mybir.SyncInfo(on_wait=[w], on_update=[]),
                            text_hint="split_wait",
                            bass_nofuse=True,
                        ))
                        n_split += 1
                    si.on_wait = waits[-max_waits:]
                out.append(inst)
            bb.instructions[:] = out
    return n_split


def _build():
    nc = bass.Bass("TRN2", target_bir_lowering=False, debug=False,
                   num_devices=NCORES)
    # Per-core inputs, already in on-chip-friendly layouts (prepared on host):
    XT_d = nc.dram_tensor("XT", [I, TB], BF16, kind="ExternalInput").ap()
    WIT_d = nc.dram_tensor("WIT", [128, NI * L], BF16, kind="ExternalInput").ap()
    BI_d = nc.dram_tensor("BI", [1, L], BF16, kind="ExternalInput").ap()
    WHT_d = nc.dram_tensor("WHT", [128, NL * L], BF16, kind="ExternalInput").ap()
    H0_d = nc.dram_tensor("H0", [128, NL * BL], BF16, kind="ExternalInput").ap()
    Y_d = nc.dram_tensor("Y", [T, BL, L], F32, kind="ExternalOutput").ap()

    with TileContext(nc) as tc:
        with (
            tc.tile_pool(name="dram", bufs=1, space="DRAM") as dramp,
            tc.tile_pool(name="const", bufs=1) as constp,
        ):
            xi_dram = dramp.tile([TB, L], BF16)

            ones_sb = constp.tile([1, 128], BF16, tag="ones")
            nc.vector.memset(ones_sb[:], 1.0)
            bi_sb = constp.tile([1, L], BF16, tag="bi")
            nc.gpsimd.dma_start(out=bi_sb[:], in_=BI_d)
            id8f = constp.tile([8, 8], F32, tag="id8f")
            nc.gpsimd.memset(id8f[:], 0.0)
            nc.gpsimd.affine_select(
                out=id8f[:], in_=id8f[:],
                compare_op=mybir.AluOpType.not_equal, fill=1.0, base=0,
                pattern=[[-1, 8]], channel_multiplier=1)
            id8 = constp.tile([8, 8], BF16, tag="id8")
            nc.vector.tensor_copy(id8[:], id8f[:])

            # ---------------- phase 1: xi = x @ Wi^T + bi ----------------
            with (
                tc.tile_pool(name="wit", bufs=1) as witp,
                tc.tile_pool(name="xt", bufs=3) as xtp,
                tc.tile_pool(name="xi1", bufs=3) as xi1p,
                tc.tile_pool(name="ps1", bufs=2, space="PSUM") as ps1p,
            ):
                wit_sb = witp.tile([128, NI * L], BF16)  # [i-part, ic*1024 + l]
                nc.gpsimd.dma_start(out=wit_sb[:], in_=WIT_d)
                xt_v = XT_d.rearrange("(c p) n -> p c n", p=128)  # [128, NI, TB]
                for mt in range(TB // 128):          # 32 row-tiles of xi
                    xt_sb = xtp.tile([128, NI * 128], BF16, tag="xt")
                    nc.gpsimd.dma_start(
                        out=xt_sb[:],
                        in_=xt_v[:, :, mt * 128:(mt + 1) * 128])
                    ps = ps1p.tile([128, L], F32, tag="ps1")
                    for half in range(2):
                        sl = slice(half * 512, (half + 1) * 512)
                        # bias as a K=1 matmul: out[tb, l] += ones[tb] * bi[l]
                        nc.tensor.matmul(ps[:, sl], ones_sb[0:1, 0:128],
                                         bi_sb[0:1, sl], start=True, stop=False)
                        for ic in range(NI):
                            nc.tensor.matmul(
                                ps[:, sl],
                                xt_sb[:, ic * 128:(ic + 1) * 128],
                                wit_sb[:, ic * L + half * 512: ic * L + half * 512 + 512],
                                start=False, stop=(ic == NI - 1))
                    xi_sb = xi1p.tile([128, L], BF16, tag="xi1")
                    nc.scalar.copy(xi_sb[:], ps[:])
                    nc.gpsimd.dma_start(
                        out=xi_dram[mt * 128:(mt + 1) * 128, :], in_=xi_sb[:])

            # ---------------- phase 2: the recurrence ----------------
            with (
                tc.tile_pool(name="wht", bufs=1) as whtp,
                tc.tile_pool(name="xiw", bufs=2) as xiwp,
                tc.tile_pool(name="hsb", bufs=3) as hsbp,
                tc.tile_pool(name="hbf", bufs=2) as hbfp,
                tc.tile_pool(name="hrhs", bufs=2) as hrhsp,
                tc.tile_pool(name="ps2", bufs=2, space="PSUM") as ps2p,
                tc.tile_pool(name="pst", bufs=2, space="PSUM") as pstp,
            ):
                wht_sb = whtp.tile([128, NL * L], BF16)  # [i-part, k*1024 + n]
                nc.gpsimd.dma_start(out=wht_sb[:], in_=WHT_d)
                h_rhs = hrhsp.tile([128, NL * BL], BF16, tag="hrhs")
                nc.gpsimd.dma_start(out=h_rhs[:], in_=H0_d)
                xi_view = xi_dram[:].rearrange("(t b) l -> b t l", b=BL)

                for w in range(T // WIN):
                    xi_win = xiwp.tile([BL, WIN, L], BF16, tag="xiw")
                    nc.gpsimd.dma_start(
                        out=xi_win[:],
                        in_=xi_view[:, w * WIN:(w + 1) * WIN, :])
                    for tl in range(WIN):
                        t = w * WIN + tl
                        ps = ps2p.tile([8, L], F32, tag="ps2")
                        for half in range(2):
                            sl = slice(half * 512, (half + 1) * 512)
                            nc.tensor.matmul(
                                ps[:, sl], id8[:],
                                xi_win[:, tl, half * 512:half * 512 + 512],
                                start=True, stop=False)
                            for k in range(NL):
                                nc.tensor.matmul(
                                    ps[:, sl],
                                    h_rhs[:, k * BL:(k + 1) * BL],
                                    wht_sb[:, k * L + half * 512: k * L + half * 512 + 512],
                                    start=False, stop=(k == NL - 1))
                        h_sb = hsbp.tile([8, L], F32, tag="hsb")
                        nc.scalar.activation(h_sb[:], ps[:],
                                             mybir.ActivationFunctionType.Tanh)
                        nc.gpsimd.dma_start(out=Y_d[t], in_=h_sb[:])
                        h_bf = hbfp.tile([8, L], BF16, tag="hbf")
                        nc.vector.tensor_copy(h_bf[:], h_sb[:])
                        pst = pstp.tile([128, NL * BL], BF16, tag="pst")
                        for c in range(NL):
                            nc.tensor.transpose(
                                pst[:, c * BL:(c + 1) * BL],
                                h_bf[:, c * 128:(c + 1) * 128], id8[:])
                        h_next = hrhsp.tile([128, NL * BL], BF16, tag="hrhs")
                        nc.vector.tensor_copy(h_next[:], pst[:])
                        h_rhs = h_next
    _legalize_sync_waits(nc)
    return nc


class _Runner:
    def __init__(self, nc, n_cores=NCORES):
        import jax
        from jax.sharding import Mesh, PartitionSpec
        from jax.experimental.shard_map import shard_map
        from concourse.bass2jax import (_bass_exec_p, install_neuronx_cc_hook,
                                        partition_id_tensor)
        install_neuronx_cc_hook()
        self.jax = jax
        self.n_cores = n_cores
        partition_name = nc.partition_id_tensor.name if nc.partition_id_tensor else None
        in_names, out_names, out_avals, zero_outs = [], [], [], []
        for alloc in nc.m.functions[0].allocations:
            if not isinstance(alloc, mybir.MemoryLocationSet):
                continue
            name = alloc.memorylocations[0].name
            if alloc.kind == "ExternalInput":
                if name != partition_name:
                    in_names.append(name)
            elif alloc.kind == "ExternalOutput":
                out_names.append(name)
                shape = tuple(alloc.tensor_shape)
                dtype = mybir.dt.np(alloc.dtype)
                out_avals.append(jax.core.ShapedArray(shape, dtype))
                zero_outs.append(np.zeros(shape, dtype))
        self.in_names, self.out_names = in_names, out_names
        self.out_avals, self.zero_outs = out_avals, zero_outs
        n_params = len(in_names)
        all_in_names = in_names + out_names
        if partition_name is not None:
            all_in_names.append(partition_name)

        def _body(*args):
            operands = list(args)
            if partition_name is not None:
                operands.append(partition_id_tensor())
            outs = _bass_exec_p.bind(
                *operands,
                out_avals=tuple(out_avals),
                in_names=tuple(all_in_names),
                out_names=tuple(out_names),
                lowering_input_output_aliases=(),
                sim_require_finite=True,
                sim_require_nnan=True,
                nc=nc,
            )
            return tuple(outs)

        donate = tuple(range(n_params, n_params + len(out_names)))
        devices = jax.devices()[:n_cores]
        mesh = Mesh(np.asarray(devices), ("core",))
        in_specs = (PartitionSpec("core"),) * (n_params + len(out_names))
        out_specs = (PartitionSpec("core"),) * len(out_names)
        self._fn = jax.jit(
            shard_map(_body, mesh=mesh, in_specs=in_specs, out_specs=out_specs,
                      check_rep=False),
            donate_argnums=donate, keep_unused=True)

    def __call__(self, in_maps):
        n = self.n_cores
        concat_in = [
            np.concatenate([np.asarray(in_maps[c][name]) for c in range(n)], axis=0)
            for name in self.in_names
        ]
        concat_zeros = [np.zeros((n * z.shape[0], *z.shape[1:]), z.dtype)
                        for z in self.zero_outs]
        out_arrs = self._fn(*concat_in, *concat_zeros)
        return [
            {name: np.asarray(out_arrs[i]).reshape(n, *self.out_avals[i].shape)[c]
             for i, name in enumerate(self.out_names)}
            for c in range(n)
        ]


_CACHE = {}


def _runner():
    if "r" not in _CACHE:
        _CACHE["r"] = _Runner(_build())
    return _CACHE["r"]


def kernel(x, h, Wi, bi, Wh):
    x = np.asarray(x, dtype=np.float32)
    h = np.asarray(h, dtype=np.float32)
    Wi = np.asarray(Wi, dtype=np.float32)
    bi = np.asarray(bi, dtype=np.float32)
    Wh = np.asarray(Wh, dtype=np.float32)

    # Shared (replicated) host-side layout prep.
    # Wi^T tiles: WIT[p, ic*L + l] = Wi[l, ic*128 + p]
    WIT = np.ascontiguousarray(
        Wi.T.reshape(NI, 128, L).transpose(1, 0, 2).reshape(128, NI * L)
    ).astype(BF16_NP)
    BI = bi.reshape(1, L).astype(BF16_NP)
    # Wh_eff = Wh.T (h_new = h @ Wh.T): WHT[p, k*L + n] = Wh.T[k*128+p, n]
    WHT = np.ascontiguousarray(
        Wh.T.reshape(NL, 128, L).transpose(1, 0, 2).reshape(128, NL * L)
    ).astype(BF16_NP)

    in_maps = []
    for c in range(NCORES):
        xs = x[:, c * BL:(c + 1) * BL, :].reshape(TB, I)  # [t*8+b, i]
        XT = np.ascontiguousarray(xs.T).astype(BF16_NP)   # [I, TB]
        hs = h[c * BL:(c + 1) * BL, :].astype(BF16_NP)    # [8, L]
        H0 = np.ascontiguousarray(
            hs.T.reshape(NL, 128, BL).transpose(1, 0, 2).reshape(128, NL * BL))
        in_maps.append({"XT": XT, "WIT": WIT, "BI": BI, "WHT": WHT, "H0": H0})

    results = _runner()(in_maps)
    y = np.empty((T, B, L), dtype=np.float32)
    for c in range(NCORES):
        y[:, c * BL:(c + 1) * BL, :] = results[c]["Y"]
    return y
